# revision 6
# baseline (speedup 1.0000x reference)
"""Multi-head attention (B=4, S=2048, D=1024, H=16) on 8 Trainium2 cores.

Sharding: core c handles batch b = c//2, head-group g = c%2 (8 heads, 512 feats).
Per core: project q/k/v (fp32r matmuls), attention in transposed-score layout
(ST[k, q] = K @ Q^T), softmax via ACT exp + bf16 mask multiply, PV with an
appended ones column (M=65) so softmax denominators fall out of the matmul,
normalize via PE outer-product broadcast + fast reciprocal, then the output
projection. Host sums the two head-group partials per batch and adds bo.
"""

import sys

sys.path.insert(0, "/opt/trn_rl_repo")
sys.path.insert(0, "/root/.axon_site/_ro/trn_rl_repo")

import numpy as np

B, S, D = 4, 2048, 1024
G = 512  # features per head-group (8 heads x 64)
N_CORES = 8

_NC_CACHE = {}


def _build_nc():
    import concourse.bacc as bacc
    import concourse.tile as tile
    from concourse import mybir

    F32 = mybir.dt.float32
    F32R = mybir.dt.float32r
    BF16 = mybir.dt.bfloat16
    AF = mybir.ActivationFunctionType
    OP = mybir.AluOpType

    nc = bacc.Bacc("TRN2", target_bir_lowering=False, debug=False, num_devices=8)

    xq = nc.dram_tensor("xq", [D, S], F32R, kind="ExternalInput")
    xk = nc.dram_tensor("xk", [D, S], F32R, kind="ExternalInput")
    xv = nc.dram_tensor("xv", [D, S], F32R, kind="ExternalInput")
    wq = nc.dram_tensor("wq", [D, G], F32R, kind="ExternalInput")
    wk = nc.dram_tensor("wk", [D, G], F32R, kind="ExternalInput")
    wv = nc.dram_tensor("wv", [D, G], F32R, kind="ExternalInput")
    wo = nc.dram_tensor("wo", [G, D], F32R, kind="ExternalInput")
    bq = nc.dram_tensor("bq", [G], F32, kind="ExternalInput")
    bk = nc.dram_tensor("bk", [G], F32, kind="ExternalInput")
    bv = nc.dram_tensor("bv", [G], F32R, kind="ExternalInput")
    maskt = nc.dram_tensor("maskt", [S, S], BF16, kind="ExternalInput")
    y = nc.dram_tensor("y", [S, D], F32, kind="ExternalOutput")

    NQ = S // 512  # 4
    NK = S // 128  # 16

    with tile.TileContext(nc) as tc:
        with (
            tc.tile_pool(name="const", bufs=1) as cpool,
            tc.tile_pool(name="res", bufs=1) as rpool,
        ):
            # small constants
            bq_sb = cpool.tile([128, 4], F32, tag="bq")
            bk_sb = cpool.tile([128, 4], F32, tag="bk")
            nc.sync.dma_start(bq_sb[:], bq.rearrange("(c p) -> p c", p=128))
            nc.sync.dma_start(bk_sb[:], bk.rearrange("(c p) -> p c", p=128))
            bv_row = cpool.tile([1, G], F32R, tag="bvr")
            nc.sync.dma_start(bv_row[:], bv.ap().unsqueeze(0))
            ones32 = cpool.tile([128, 128], F32, tag="ones32")
            nc.vector.memset(ones32[:], 1.0)
            onr = cpool.tile([128, 128], F32R, tag="onr")
            nc.scalar.activation(onr[:], ones32[:], AF.Copy)
            bvb_sb = cpool.tile([128, G], F32, tag="bvb")

            # resident activations (live across phases 1-2)
            kt_sb = rpool.tile([128, 4, S], F32R, tag="kt")
            qt_sb = rpool.tile([128, 4, S], F32R, tag="qt")
            v_ext = rpool.tile([128, NK, 8, 65], F32R, tag="vext")

            nc.scalar.activation(
                v_ext[:, :, :, 64],
                ones32[:].rearrange("p (a b) -> p a b", a=NK),
                AF.Copy,
            )

            # ============ phases 0-1: weights + projections ============
            with (
                tc.tile_pool(name="wts", bufs=1) as wt,
                tc.tile_pool(name="stage", bufs=2) as spool,
                tc.tile_pool(name="psA", bufs=4, space="PSUM") as psA,
                tc.tile_pool(name="psV", bufs=1, space="PSUM") as psV,
            ):
                wq_sb = wt.tile([128, 8, G], F32R, tag="wq")
                wk_sb = wt.tile([128, 8, G], F32R, tag="wk")
                wv_sb = wt.tile([128, 8, G], F32R, tag="wv")
                nc.sync.dma_start(wq_sb[:], wq.rearrange("(c p) g -> p c g", p=128))
                nc.sync.dma_start(wk_sb[:], wk.rearrange("(c p) g -> p c g", p=128))
                nc.sync.dma_start(wv_sb[:], wv.rearrange("(c p) g -> p c g", p=128))

                bvb_ps = psV.tile([128, G], F32, tag="bvb")
                nc.tensor.matmul(bvb_ps[:], onr[0:1, 0:128], bv_row[:])
                nc.vector.tensor_copy(bvb_sb[:], bvb_ps[:])

                for n in range(NQ):
                    sl = slice(512 * n, 512 * (n + 1))
                    stg_k = spool.tile([128, 8, 512], F32R, tag="stage")
                    nc.sync.dma_start(
                        stg_k[:], xk.rearrange("(c p) s -> p c s", p=128)[:, :, sl]
                    )
                    for f in range(4):
                        pj = psA.tile([128, 512], F32, tag="pj")
                        for c in range(8):
                            nc.tensor.matmul(
                                pj[:],
                                wk_sb[:, c, 128 * f : 128 * (f + 1)],
                                stg_k[:, c, :],
                                start=(c == 0),
                                stop=(c == 7),
                            )
                        nc.scalar.activation(
                            kt_sb[:, f, sl],
                            pj[:],
                            AF.Identity,
                            bias=bk_sb[:, f : f + 1],
                        )
                    stg_q = spool.tile([128, 8, 512], F32R, tag="stage")
                    nc.sync.dma_start(
                        stg_q[:], xq.rearrange("(c p) s -> p c s", p=128)[:, :, sl]
                    )
                    for f in range(4):
                        pj = psA.tile([128, 512], F32, tag="pj")
                        for c in range(8):
                            nc.tensor.matmul(
                                pj[:],
                                wq_sb[:, c, 128 * f : 128 * (f + 1)],
                                stg_q[:, c, :],
                                start=(c == 0),
                                stop=(c == 7),
                            )
                        nc.scalar.activation(
                            qt_sb[:, f, sl],
                            pj[:],
                            AF.Identity,
                            bias=bq_sb[:, f : f + 1],
                        )
                    stg_v = spool.tile([128, 8, 512], F32R, tag="stage")
                    nc.sync.dma_start(
                        stg_v[:], xv.rearrange("(c p) s -> p c s", p=128)[:, :, sl]
                    )
                    for ssub in range(4):
                        sc = 4 * n + ssub
                        pj = psA.tile([128, 512], F32, tag="pj")
                        for c in range(8):
                            nc.tensor.matmul(
                                pj[:],
                                stg_v[:, c, 128 * ssub : 128 * (ssub + 1)],
                                wv_sb[:, c, :],
                                start=(c == 0),
                                stop=(c == 7),
                            )
                        nc.vector.tensor_tensor(
                            out=v_ext[:, sc, :, 0:64],
                            in0=pj[:].rearrange("p (h d) -> p h d", h=8),
                            in1=bvb_sb[:].rearrange("p (h d) -> p h d", h=8),
                            op=OP.add,
                        )

            # ============ phases 2-3: attention + output proj ============
            with (
                tc.tile_pool(name="res2", bufs=1) as r2,
                tc.tile_pool(name="work", bufs=3) as wpool,
                tc.tile_pool(name="norm", bufs=2) as npool,
            ):
                ctxn = r2.tile([128, 4, S], F32R, tag="ctxn")
                wo_sb = r2.tile([128, 4, D], F32R, tag="wo")
                nc.sync.dma_start(wo_sb[:], wo.rearrange("(c p) o -> p c o", p=128))

                with (
                    tc.tile_pool(name="psB", bufs=2, space="PSUM") as psB,
                    tc.tile_pool(name="psC", bufs=4, space="PSUM") as psC,
                ):
                    for qpass in range(2):  # head quads
                        for qc in range(NQ):
                            qsl = slice(512 * qc, 512 * (qc + 1))
                            ctx_tiles = [
                                psC.tile(
                                    [65, 512],
                                    F32,
                                    tag="ctx",
                                    name=f"ctx_{qpass}_{qc}_{i}",
                                )
                                for i in range(4)
                            ]
                            for kc in range(NK):
                                ksl = slice(128 * kc, 128 * (kc + 1))
                                mk = wpool.tile([128, 512], BF16, tag="mk")
                                nc.sync.dma_start(mk[:], maskt[ksl, qsl])
                                for pr in range(2):
                                    f = 2 * qpass + pr
                                    st = psB.tile([128, 2, 512], F32, tag="st")
                                    nc.tensor.matmul(
                                        st[:, 0, :],
                                        kt_sb[0:64, f, ksl],
                                        qt_sb[0:64, f, qsl],
                                    )
                                    nc.tensor.matmul(
                                        st[:, 1, :],
                                        kt_sb[64:128, f, ksl],
                                        qt_sb[64:128, f, qsl],
                                    )
                                    e2 = wpool.tile(
                                        [128, 2, 512], F32, tag="e2", bufs=2
                                    )
                                    nc.scalar.activation(
                                        e2[:], st[:], AF.Exp, scale=0.125
                                    )
                                    p2 = wpool.tile(
                                        [128, 2, 512], F32R, tag="p2", bufs=2
                                    )
                                    eng = (
                                        nc.vector
                                        if (kc + pr) % 2 == 0
                                        else nc.gpsimd
                                    )
                                    eng.tensor_tensor(
                                        out=p2[:],
                                        in0=e2[:],
                                        in1=mk[:]
                                        .unsqueeze(1)
                                        .broadcast_to([128, 2, 512]),
                                        op=OP.mult,
                                    )
                                    for j in range(2):
                                        nc.tensor.matmul(
                                            ctx_tiles[2 * pr + j][:],
                                            v_ext[:, kc, 4 * qpass + 2 * pr + j, :],
                                            p2[:, j, :],
                                            start=(kc == 0),
                                            stop=(kc == NK - 1),
                                        )
                            for hq in range(4):
                                h = 4 * qpass + hq
                                cs = npool.tile([65, 512], F32R, tag="cs")
                                nc.vector.tensor_copy(cs[:], ctx_tiles[hq][:])
                                dnb = psB.tile([64, 512], F32, tag="st")
                                nc.tensor.matmul(
                                    dnb[:],
                                    onr[64:65, 0:64],
                                    cs[64:65, :],
                                    tile_position=(64, 0),
                                )
                                dnb_sb = npool.tile(
                                    [64, 512], F32, tag="dnsb", bufs=1
                                )
                                nc.vector.tensor_copy(dnb_sb[:], dnb[:])
                                rec = npool.tile([64, 512], F32, tag="rec")
                                nc.vector.reciprocal_approx_fast(rec[:], dnb_sb[:])
                                if h % 2 == 0:
                                    nc.vector.tensor_tensor(
                                        out=ctxn[0:64, h // 2, qsl],
                                        in0=cs[0:64, :],
                                        in1=rec[:],
                                        op=OP.mult,
                                    )
                                else:
                                    odd = npool.tile([64, 512], F32R, tag="odd")
                                    nc.vector.tensor_tensor(
                                        out=odd[:],
                                        in0=cs[0:64, :],
                                        in1=rec[:],
                                        op=OP.mult,
                                    )
                                    nc.sync.dma_start(
                                        ctxn[64:128, h // 2, qsl], odd[:]
                                    )

                with tc.tile_pool(name="psY", bufs=2, space="PSUM") as psY:
                    for sp in range(8):
                        yps = psY.tile([128, 4, 512], F32, tag="yps")
                        for sci in range(2):
                            sc = 2 * sp + sci
                            ssl = slice(128 * sc, 128 * (sc + 1))
                            for oc in range(2):
                                for dvc in range(4):
                                    nc.tensor.matmul(
                                        yps[:, 2 * sci + oc, :],
                                        ctxn[:, dvc, ssl],
                                        wo_sb[:, dvc, 512 * oc : 512 * (oc + 1)],
                                        start=(dvc == 0),
                                        stop=(dvc == 3),
                                    )
                        y_sb = wpool.tile([128, 4, 512], F32, tag="ysb", bufs=2)
                        nc.scalar.activation(y_sb[:], yps[:], AF.Copy)
                        for sci in range(2):
                            sc = 2 * sp + sci
                            nc.sync.dma_start(
                                y[128 * sc : 128 * (sc + 1), :],
                                y_sb[:, 2 * sci : 2 * sci + 2, :].rearrange(
                                    "p a b -> p (a b)"
                                ),
                            )

    nc.compile()
    return nc


def kernel(q, k, v, mask, Wq, bq, Wk, bk, Wv, bv, Wo, bo):
    import ml_dtypes
    from concourse.bass_utils import run_bass_kernel_spmd

    if "nc" not in _NC_CACHE:
        _NC_CACHE["nc"] = _build_nc()
    nc = _NC_CACHE["nc"]

    q, k, v = np.asarray(q), np.asarray(k), np.asarray(v)
    mask = np.asarray(mask)
    Wq, Wk, Wv, Wo = (np.asarray(a, np.float32) for a in (Wq, Wk, Wv, Wo))
    bq, bk, bv, bo = (np.asarray(a, np.float32) for a in (bq, bk, bv, bo))

    in_maps = []
    xt = {}
    mt = {}
    for b in range(B):
        xt[b] = (
            np.ascontiguousarray(q[b].T, np.float32),
            np.ascontiguousarray(k[b].T, np.float32),
            np.ascontiguousarray(v[b].T, np.float32),
        )
        mt[b] = np.ascontiguousarray(mask[b].T).astype(ml_dtypes.bfloat16)
    for c in range(N_CORES):
        b, g = c // 2, c % 2
        sl = slice(G * g, G * (g + 1))
        in_maps.append(
            {
                "xq": xt[b][0],
                "xk": xt[b][1],
                "xv": xt[b][2],
                "wq": np.ascontiguousarray(Wq[sl, :].T),
                "wk": np.ascontiguousarray(Wk[sl, :].T),
                "wv": np.ascontiguousarray(Wv[sl, :].T),
                "wo": np.ascontiguousarray(Wo[:, sl].T),
                "bq": np.ascontiguousarray(bq[sl]),
                "bk": np.ascontiguousarray(bk[sl]),
                "bv": np.ascontiguousarray(bv[sl]),
                "maskt": mt[b],
            }
        )

    _NC_CACHE["last_in_maps"] = in_maps
    res = run_bass_kernel_spmd(nc, in_maps, list(range(N_CORES)))
    out = np.empty((B, S, D), np.float32)
    for b in range(B):
        out[b] = res.results[2 * b]["y"] + res.results[2 * b + 1]["y"] + bo
    return out
